# revision 12
# baseline (speedup 1.0000x reference)
# Multi-head attention block (projections + softmax attention + output
# projection + residual + LayerNorm) for Trainium2, 8 NeuronCores.
#
# Sharding: data-parallel. 8 cores = 4 batches x 2 query-halves. Core c
# handles batch c//2, query rows (c%2)*1024 .. +1024, with the full K/V
# of its batch. No cross-core communication.
#
# All matmuls run as fp8 DoubleRow (2 stacked contraction planes per PE
# cell, 0.5 cycles/column): projections contract e=1024 as 4 DR matmuls
# of 2x128, scores contract the 64-dim head as one DR matmul of 2x32,
# attnV contracts 256 keys (two 128-key tiles) per DR matmul. Inputs
# arrive pre-transposed / pre-quantized from the host as uint8 (fp8
# bytes), so the kernel does no PE transposes at all.
#
# Softmax exp is split across three engines: Act runs true exp into
# e5m2; Pool and DVE synthesize e5m2 bytes directly with one
# tensor_scalar each (bits(e^x) ~= uint8(x*4*log2e + 60.5), exact to
# ~5% which is inside the fp8 noise floor).
#
# Self-contained: hardcodes all shapes from the problem spec.
#   B, S, D, H = 4, 2048, 1024, 16 ; head_dim = 64 ; eps = 1e-6

from contextlib import ExitStack

import ml_dtypes
import numpy as np

import concourse.bass as bass
import concourse.mybir as mybir
import concourse.tile as tile
from concourse import bacc
from concourse.bass_utils import run_bass_kernel_spmd

B, S, D, H = 4, 2048, 1024, 16
HD = D // H          # 64 head dim
EPS = 1e-6
NCORES = 8
SQ = (B * S) // NCORES   # 1024 query rows per core
SK = S                   # 2048 keys per core
P = 128

ET = D // P     # 8  e (input-feature) tiles
UT = ET // 2    # 4  DoubleRow e-pair blocks (256-deep each)
DT = D // P     # 8  d (output-feature) tiles
IT = SQ // P    # 8  query row-tiles
JT = SK // P    # 16 key row-tiles
JP = JT // 2    # 8  key tile-pairs (256 keys per attnV DR matmul)
IC = SQ // 512  # 2  query 512-chunks
JC = SK // 512  # 4  key 512-chunks

FP32 = mybir.dt.float32
U8 = mybir.dt.uint8
F8E4 = mybir.dt.float8e4
F8E5 = mybir.dt.float8e5
DR = mybir.MatmulPerfMode.DoubleRow

# bits(e^x) as e5m2 ~= x * 4*log2(e) + 60.5; the extra 0.125 folds in the
# 1/sqrt(64) score scale. Constant-factor bias cancels in softmax.
E5_SCALE = 4.0 * 1.4426950408889634 * 0.125
E5_BIAS = 60.5
ONE_E5M2 = 60  # byte pattern of 1.0 in e5m2

# exp engine split per (head, jt) unit. GPSIMD/Pool cannot touch PSUM, so
# only Act (true exp) and DVE (e5m2 bit-trick) share the work; Act is
# faster per element but also carries half the evacuation copies. 9/7.
_DVE_R = (1, 3, 5, 8, 10, 12, 14)


def _exp_engine(i):
    return "dve" if (i % 16) in _DVE_R else "act"


def _emit(tc: tile.TileContext, ctx: ExitStack):
    nc = tc.nc

    QT = nc.dram_tensor("QT8", [D, SQ], U8, kind="ExternalInput").ap()
    KT = nc.dram_tensor("KT8", [D, SK], U8, kind="ExternalInput").ap()
    VT = nc.dram_tensor("VT8", [D, SK], U8, kind="ExternalInput").ap()
    WQ = nc.dram_tensor("Wq8", [D, D], U8, kind="ExternalInput").ap()
    WK = nc.dram_tensor("Wk8", [D, D], U8, kind="ExternalInput").ap()
    WV = nc.dram_tensor("Wv8", [D, D], U8, kind="ExternalInput").ap()
    WO = nc.dram_tensor("Wo8", [D, D], U8, kind="ExternalInput").ap()
    QR = nc.dram_tensor("Qres", [SQ, D], FP32, kind="ExternalInput").ap()
    gamma = nc.dram_tensor("ln_gamma", [D], FP32, kind="ExternalInput").ap()
    beta = nc.dram_tensor("ln_beta", [D], FP32, kind="ExternalInput").ap()
    out = nc.dram_tensor("out", [SQ, D], FP32, kind="ExternalOutput").ap()

    persist = ctx.enter_context(tc.tile_pool(name="persist", bufs=1))

    # ---- persistent SBUF ----
    qin = persist.tile([P, ET, SQ], U8, tag="qin", name="qin")
    kin = persist.tile([P, ET, SK], U8, tag="kin", name="kin")
    vin = persist.tile([P, ET, SK], U8, tag="vin", name="vin")
    wq = persist.tile([P, ET, D], U8, tag="wq", name="wq")
    wk = persist.tile([P, ET, D], U8, tag="wk", name="wk")
    wv = persist.tile([P, ET, D], U8, tag="wv", name="wv")
    wo = persist.tile([P, ET, D], U8, tag="wo", name="wo")
    # q/k in scores-DR layout: per d-tile, heads 2dt / 2dt+1 live on
    # partitions [0:64) / [64:128), t=0 holds the head's 64 dims, t=1 is
    # zeros (DR needs a second contraction plane; half-wasted but the
    # stream still runs at 0.5 cycles/column and evac copies stay
    # full-width).
    QS = [persist.tile([P, 2, SQ], U8, tag=f"qs{g}", name=f"qs{g}") for g in range(DT)]
    KS = [persist.tile([P, 2, SK], U8, tag=f"ks{g}", name=f"ks{g}") for g in range(DT)]
    # v in attnV-DR layout: per key-tile-pair, [j-in-tile, head, tile-parity,
    # 64 dims] (e5m2)
    v2 = [
        persist.tile([P, H, 2, HD], U8, tag=f"v{jp}", name=f"v{jp}")
        for jp in range(JP)
    ]
    # e5m2 ones, lhsT of the denominator matmul: l_bc[d, i] = sum_j expt
    ones_t = persist.tile([P, 2, HD], U8, tag="ones_t", name="ones_t")
    # normalized attention output [d, i] (fp8e4) = O-proj lhsT
    attnT = persist.tile([P, ET, SQ], U8, tag="attnT", name="attnT")

    gamma_b = persist.tile([P, D], FP32, tag="gamma_b", name="gamma_b")
    nc.gpsimd.dma_start(out=gamma_b[:], in_=gamma[None, :].to_broadcast((P, D)))
    beta_b = persist.tile([P, D], FP32, tag="beta_b", name="beta_b")
    nc.gpsimd.dma_start(out=beta_b[:], in_=beta[None, :].to_broadcast((P, D)))
    eps_t = persist.tile([P, 1], FP32, tag="eps_t", name="eps_t")
    nc.vector.memset(eps_t[:], EPS)

    # ---- input DMA ----
    for et in range(ET):
        sl = slice(et * P, (et + 1) * P)
        nc.sync.dma_start(out=wk[:, et, :], in_=WK[sl, :])
        nc.sync.dma_start(out=kin[:, et, :], in_=KT[sl, :])
        nc.sync.dma_start(out=wq[:, et, :], in_=WQ[sl, :])
        nc.sync.dma_start(out=qin[:, et, :], in_=QT[sl, :])
        nc.sync.dma_start(out=wv[:, et, :], in_=WV[sl, :])
        nc.sync.dma_start(out=vin[:, et, :], in_=VT[sl, :])
        nc.sync.dma_start(out=wo[:, et, :], in_=WO[sl, :])
    nc.gpsimd.memset(ones_t[:], ONE_E5M2)
    # zero the unused second contraction plane of the scores operands
    for dt in range(DT):
        nc.gpsimd.memset(KS[dt][:, 1, :], 0)
        nc.vector.memset(QS[dt][:, 1, :], 0)

    # ---- projections (fp8 DoubleRow, contraction e=1024 as 4x 2x128) ----
    with tc.tile_pool(name="pj", bufs=2, space="PSUM") as pjp:
        # K-proj: out [d-tile, j] -> KS
        for dt in range(DT):
            for jch in range(JC):
                ps = pjp.tile([P, 512], FP32, tag="pj", name="pj")
                for u in range(UT):
                    nc.tensor.matmul(
                        ps[:],
                        wk[:, 2 * u : 2 * u + 2, dt * P : (dt + 1) * P].bitcast(F8E4),
                        kin[:, 2 * u : 2 * u + 2, jch * 512 : (jch + 1) * 512].bitcast(
                            F8E4
                        ),
                        start=(u == 0),
                        stop=(u == UT - 1),
                        perf_mode=DR,
                    )
                jsl = slice(jch * 512, (jch + 1) * 512)
                if jch % 2 == 0:
                    nc.scalar.copy(out=KS[dt][:, 0, jsl].bitcast(F8E4), in_=ps[:])
                else:
                    nc.vector.tensor_copy(
                        out=KS[dt][:, 0, jsl].bitcast(F8E4), in_=ps[:]
                    )
        # Q-proj: out [d-tile, i] -> QS
        for dt in range(DT):
            for icc in range(IC):
                ps = pjp.tile([P, 512], FP32, tag="pj", name="pj")
                for u in range(UT):
                    nc.tensor.matmul(
                        ps[:],
                        wq[:, 2 * u : 2 * u + 2, dt * P : (dt + 1) * P].bitcast(F8E4),
                        qin[:, 2 * u : 2 * u + 2, icc * 512 : (icc + 1) * 512].bitcast(
                            F8E4
                        ),
                        start=(u == 0),
                        stop=(u == UT - 1),
                        perf_mode=DR,
                    )
                isl = slice(icc * 512, (icc + 1) * 512)
                if icc % 2 == 0:
                    nc.scalar.copy(out=QS[dt][:, 0, isl].bitcast(F8E4), in_=ps[:])
                else:
                    nc.vector.tensor_copy(
                        out=QS[dt][:, 0, isl].bitcast(F8E4), in_=ps[:]
                    )
        # V-proj: out [j-tile, d] -> v2 (e5m2)
        for jt in range(JT):
            for dc in range(2):
                ps = pjp.tile([P, 512], FP32, tag="pj", name="pj")
                for u in range(UT):
                    nc.tensor.matmul(
                        ps[:],
                        vin[:, 2 * u : 2 * u + 2, jt * P : (jt + 1) * P].bitcast(F8E4),
                        wv[:, 2 * u : 2 * u + 2, dc * 512 : (dc + 1) * 512].bitcast(
                            F8E4
                        ),
                        start=(u == 0),
                        stop=(u == UT - 1),
                        perf_mode=DR,
                    )
                if jt % 2 == 0:
                    nc.scalar.copy(
                        out=v2[jt // 2][:, dc * 8 : (dc + 1) * 8, jt % 2, :].bitcast(
                            F8E5
                        ),
                        in_=ps[:].rearrange("p (h d) -> p h d", d=HD),
                    )
                else:
                    nc.vector.tensor_copy(
                        out=v2[jt // 2][:, dc * 8 : (dc + 1) * 8, jt % 2, :].bitcast(
                            F8E5
                        ),
                        in_=ps[:].rearrange("p (h d) -> p h d", d=HD),
                    )

    # ---- attention ----
    attn_ctx = ExitStack()
    psp = attn_ctx.enter_context(tc.tile_pool(name="ps_s", bufs=2, space="PSUM"))
    pop = attn_ctx.enter_context(tc.tile_pool(name="po", bufs=2, space="PSUM"))
    plp = attn_ctx.enter_context(tc.tile_pool(name="pl", bufs=2, space="PSUM"))
    exptp = attn_ctx.enter_context(tc.tile_pool(name="expt", bufs=4))
    normp = attn_ctx.enter_context(tc.tile_pool(name="norm", bufs=4))

    expi = 0
    for h in range(H):
        hdt = h // 2
        hsl = slice(64 * (h % 2), 64 * (h % 2) + 64)
        po = [pop.tile([HD, 512], FP32, tag="po", name="po") for _ in range(IC)]
        lb = [plp.tile([HD, 512], FP32, tag="lb", name="lb") for _ in range(IC)]
        for jp in range(JP):
            et2 = exptp.tile([P, 2, SQ], U8, tag="expt", name="expt")
            for t in range(2):
                jt = 2 * jp + t
                ps = psp.tile([P, SQ], FP32, tag="ps", name="ps")
                for icc in range(IC):
                    nc.tensor.matmul(
                        ps[:, icc * 512 : (icc + 1) * 512],
                        KS[hdt][hsl, :, jt * P : (jt + 1) * P].bitcast(F8E4),
                        QS[hdt][hsl, :, icc * 512 : (icc + 1) * 512].bitcast(F8E4),
                        start=True,
                        stop=True,
                        perf_mode=DR,
                    )
                eng = _exp_engine(expi)
                expi += 1
                if eng == "act":
                    nc.scalar.activation(
                        out=et2[:, t, :].bitcast(F8E5),
                        in_=ps[:],
                        func=mybir.ActivationFunctionType.Exp,
                        scale=0.125,
                    )
                else:
                    e = nc.gpsimd if eng == "pool" else nc.vector
                    e.tensor_scalar(
                        out=et2[:, t, :],
                        in0=ps[:],
                        scalar1=E5_SCALE,
                        scalar2=E5_BIAS,
                        op0=mybir.AluOpType.mult,
                        op1=mybir.AluOpType.add,
                    )
            for icc in range(IC):
                esl = et2[:, :, icc * 512 : (icc + 1) * 512].bitcast(F8E5)
                nc.tensor.matmul(
                    po[icc][:],
                    v2[jp][:, h, :, :].bitcast(F8E5),
                    esl,
                    start=(jp == 0),
                    stop=(jp == JP - 1),
                    perf_mode=DR,
                )
                # denominator, broadcast across all 64 output partitions
                nc.tensor.matmul(
                    lb[icc][:],
                    ones_t[:].bitcast(F8E5),
                    esl,
                    start=(jp == 0),
                    stop=(jp == JP - 1),
                    perf_mode=DR,
                )
        # normalize by the softmax denominator (lb, already partition-broadcast)
        for icc in range(IC):
            isl = slice(icc * 512, (icc + 1) * 512)
            rlb = normp.tile([HD, 512], FP32, tag="rlb", name="rlb")
            nc.vector.reciprocal_approx_fast(out=rlb[:], in_=lb[icc][:])
            if h % 2 == 0:
                nc.vector.tensor_mul(
                    out=attnT[0:HD, h // 2, isl].bitcast(F8E4),
                    in0=po[icc][:],
                    in1=rlb[:],
                )
            else:
                tmp = normp.tile([HD, 512], U8, tag="tmp", name="tmp")
                nc.vector.tensor_mul(
                    out=tmp[:].bitcast(F8E4), in0=po[icc][:], in1=rlb[:]
                )
                nc.sync.dma_start(out=attnT[HD:P, h // 2, isl], in_=tmp[:])

    attn_ctx.close()

    # ---- output projection + residual + LayerNorm ----
    with (
        tc.tile_pool(name="pf", bufs=2, space="PSUM") as pfp,
        tc.tile_pool(name="ln", bufs=3) as lnp,
        tc.tile_pool(name="stage", bufs=3) as stp,
    ):
        for it in range(IT):
            rq = stp.tile([P, D], FP32, tag="rq", name="rq")
            nc.sync.dma_start(out=rq[:], in_=QR[it * P : (it + 1) * P, :])
            f = lnp.tile([P, D], FP32, tag="f", name="f")
            for ecc in range(IC):
                pf = pfp.tile([P, 512], FP32, tag="pf", name="pf")
                for u in range(UT):
                    nc.tensor.matmul(
                        pf[:],
                        attnT[:, 2 * u : 2 * u + 2, it * P : (it + 1) * P].bitcast(
                            F8E4
                        ),
                        wo[:, 2 * u : 2 * u + 2, ecc * 512 : (ecc + 1) * 512].bitcast(
                            F8E4
                        ),
                        start=(u == 0),
                        stop=(u == UT - 1),
                        perf_mode=DR,
                    )
                esl = slice(ecc * 512, (ecc + 1) * 512)
                if ecc % 2 == 0:
                    nc.scalar.activation(
                        out=f[:, esl],
                        in_=pf[:],
                        func=mybir.ActivationFunctionType.Copy,
                        bias=0.0,
                    )
                    nc.gpsimd.tensor_add(out=f[:, esl], in0=f[:, esl], in1=rq[:, esl])
                else:
                    nc.vector.tensor_add(out=f[:, esl], in0=pf[:], in1=rq[:, esl])
            stats = lnp.tile([P, 2, 6], FP32, tag="stats", name="stats")
            fv = f[:].rearrange("p (s x) -> p s x", s=2)
            for s_ in range(2):
                nc.vector.bn_stats(out=stats[:, s_, :], in_=fv[:, s_, :])
            mv = lnp.tile([P, 2], FP32, tag="mv", name="mv")
            nc.vector.bn_aggr(out=mv[:], in_=stats[:])
            rstd = lnp.tile([P, 1], FP32, tag="rstd", name="rstd")
            nc.scalar.activation(
                out=rstd[:],
                in_=mv[:, 1:2],
                func=mybir.ActivationFunctionType.Sqrt,
                bias=eps_t[:],
                scale=1.0,
            )
            nc.vector.reciprocal(out=rstd[:], in_=rstd[:])
            o_sb = lnp.tile([P, D], FP32, tag="o", name="o")
            nc.gpsimd.tensor_scalar(
                out=o_sb[:],
                in0=f[:],
                scalar1=mv[:, 0:1],
                scalar2=rstd[:],
                op0=mybir.AluOpType.subtract,
                op1=mybir.AluOpType.mult,
            )
            nc.gpsimd.tensor_mul(out=o_sb[:], in0=o_sb[:], in1=gamma_b[:])
            nc.gpsimd.tensor_add(out=o_sb[:], in0=o_sb[:], in1=beta_b[:])
            nc.sync.dma_start(out=out[it * P : (it + 1) * P, :], in_=o_sb[:])


_CACHE = {}


def build_program():
    if "nc" not in _CACHE:
        nc = bacc.Bacc(
            "TRN2",
            target_bir_lowering=False,
            debug=False,
            enable_asserts=False,
            num_devices=NCORES,
        )
        with tile.TileContext(nc) as tc, ExitStack() as ctx:
            _emit(tc, ctx)
        nc.compile()
        _CACHE["nc"] = nc
    return _CACHE["nc"]


def shard_inputs(inputs):
    f8 = ml_dtypes.float8_e4m3

    def to8T(x):
        # [rows, D] fp32 -> transposed fp8 bytes [D, rows]
        return np.ascontiguousarray(np.asarray(x, np.float32).T).astype(f8).view(
            np.uint8
        )

    Q = np.asarray(inputs["Q"], np.float32)
    K = np.asarray(inputs["K"], np.float32)
    V = np.asarray(inputs["V"], np.float32)
    wq8 = to8T(inputs["Wq"])  # [e, d]
    wk8 = to8T(inputs["Wk"])
    wv8 = to8T(inputs["Wv"])
    wo8 = np.ascontiguousarray(np.asarray(inputs["Wo"], np.float32).T).astype(
        f8
    ).view(np.uint8)  # [d, e]
    gam = np.ascontiguousarray(np.asarray(inputs["ln_gamma"], np.float32))
    bet = np.ascontiguousarray(np.asarray(inputs["ln_beta"], np.float32))

    kt8 = [to8T(K[b]) for b in range(B)]
    vt8 = [to8T(V[b]) for b in range(B)]

    in_maps = []
    for c in range(NCORES):
        b, hf = c // 2, c % 2
        rows = slice(hf * SQ, (hf + 1) * SQ)
        in_maps.append(
            {
                "QT8": to8T(Q[b, rows]),
                "KT8": kt8[b],
                "VT8": vt8[b],
                "Wq8": wq8,
                "Wk8": wk8,
                "Wv8": wv8,
                "Wo8": wo8,
                "Qres": np.ascontiguousarray(Q[b, rows]),
                "ln_gamma": gam,
                "ln_beta": bet,
            }
        )
    return in_maps


def unshard_outputs(results):
    full = np.zeros((B, S, D), np.float32)
    for c in range(NCORES):
        b, hf = c // 2, c % 2
        full[b, hf * SQ : (hf + 1) * SQ, :] = results[c]["out"]
    return full


def kernel(**inputs):
    nc = build_program()
    in_maps = shard_inputs(inputs)
    res = run_bass_kernel_spmd(nc, in_maps, list(range(NCORES)))
    return unshard_outputs(res.results)


if __name__ == "__main__":
    rng = np.random.default_rng(0)
    ins = {
        "Q": rng.standard_normal((B, S, D)).astype(np.float32),
        "K": rng.standard_normal((B, S, D)).astype(np.float32),
        "V": rng.standard_normal((B, S, D)).astype(np.float32),
        "Wq": (rng.standard_normal((D, D)) / np.sqrt(D)).astype(np.float32),
        "Wk": (rng.standard_normal((D, D)) / np.sqrt(D)).astype(np.float32),
        "Wv": (rng.standard_normal((D, D)) / np.sqrt(D)).astype(np.float32),
        "Wo": (rng.standard_normal((D, D)) / np.sqrt(D)).astype(np.float32),
        "ln_gamma": np.ones(D, np.float32),
        "ln_beta": np.zeros(D, np.float32),
    }
    out = kernel(**ins)
    print(out.shape, out.dtype, np.abs(out).max())
